# revision 1
# baseline (speedup 1.0000x reference)
"""GCN encoder (5-layer GCNConv + global mean pool) on 8 Trainium2 NeuronCores.

Strategy (node sharding):
  - 10000 nodes split contiguously across 8 cores (1250/core, padded to 1280).
  - Each layer: per-core GEMM (h @ W, fp16 operands, fp32 PSUM) ->
    AllGather of the fp16 hW slices into pair-shared DRAM ->
    dma_gather of per-edge source rows (dst-sorted, chunked 128 edges) ->
    segment-sum as one-hot matmul per chunk (Seg[128e,128d].T @ msgs[128e,fo]
    accumulated in PSUM per 128-dst tile; GCN norm folded into Seg values) ->
    bias + relu (DVE) -> PE transpose to keep h^T for the next GEMM.
  - Mean-pool as matmul with 1/count one-hot, AllReduce over cores.

The graph structure (edge sort, one-hot Seg with norm, gather indices,
pool matrix) is preprocessed on host; all FLOPs on x/W run on device.
"""
import sys

import numpy as np

sys.path.insert(0, "/opt/trn_rl_repo")

import concourse.bacc as bacc
import concourse.bass as bass  # noqa: F401
import concourse.mybir as mybir
import concourse.tile as tile
from concourse import bass_utils

dt = mybir.dt

N = 10000
E = 150000
G = 64
DIN = 128
DHID = 512
DOUT = 128
C = 8
NPC = N // C          # 1250 nodes per core
NTILE = 10            # ceil(1250/128)
NPAD = NTILE * 128    # 1280 padded rows per core
NROWS = C * NPAD      # 10240 rows in the allgathered table
FO = [DHID, DHID, DHID, DHID, DOUT]
FIT = [1, 4, 4, 4, 4]  # fi tiles per layer (fi=128 for L1, 512 for L2-5)


def _preprocess(edge_index, batch):
    """Build per-core gather indices, one-hot Seg blocks, and pool matrix."""
    src = np.concatenate([edge_index[0], np.arange(N, dtype=np.int64)])
    dst = np.concatenate([edge_index[1], np.arange(N, dtype=np.int64)])
    deg = np.bincount(dst, minlength=N).astype(np.float64)
    dinv = np.where(deg > 0, 1.0 / np.sqrt(deg), 0.0)
    norm = (dinv[src] * dinv[dst]).astype(np.float32)

    core = dst // NPC
    t_of = (dst % NPC) // 128
    dloc = (dst % NPC) % 128

    # edge counts per (core, tile) -> uniform chunk budget T_pad
    cnt = np.zeros((C, NTILE), np.int64)
    np.add.at(cnt, (core, t_of), 1)
    t_pad = int(np.ceil(cnt.max() / 128))
    nchunk = NTILE * t_pad

    # slot of each edge inside its (core, tile) bucket
    order = np.lexsort((dst, t_of, core))
    s_src, s_core, s_t, s_dloc, s_norm = (
        src[order], core[order], t_of[order], dloc[order], norm[order])
    # position within bucket
    bucket = s_core * NTILE + s_t
    start = np.zeros(C * NTILE, np.int64)
    start[1:] = np.cumsum(np.bincount(bucket, minlength=C * NTILE))[:-1]
    pos = np.arange(len(s_src)) - start[bucket]

    chunk = s_t * t_pad + pos // 128     # chunk id within core
    erow = pos % 128                     # row within chunk

    # padded-row index of each source node in the allgathered table
    srow = (s_src // NPC) * NPAD + (s_src % NPC)

    gidx = np.zeros((C, nchunk * 128), np.int16)
    seg = np.zeros((C, 128, nchunk, 128), np.float16)
    gidx[s_core, chunk * 128 + erow] = srow.astype(np.int16)
    seg[s_core, erow, chunk, s_dloc] = s_norm.astype(np.float16)

    # idx wrap: logical idx i -> partition i%16, column i//16; replicate x8
    gidx_w = np.ascontiguousarray(
        np.tile(gidx.reshape(C, -1, 16).transpose(0, 2, 1), (1, 8, 1)))

    # pool matrix [C, 128, NTILE, G]: 1/count at (node row, graph)
    gcnt = np.bincount(batch, minlength=G).astype(np.float64)
    inv = (1.0 / np.maximum(gcnt, 1.0))
    pool = np.zeros((C, 128, NTILE, G), np.float16)
    nodes = np.arange(N)
    pc, pr = nodes // NPC, nodes % NPC
    pool[pc, pr % 128, pr // 128, batch] = inv[batch].astype(np.float16)

    return gidx_w, seg, pool, t_pad, nchunk


def _build(t_pad, nchunk):
    nc = bacc.Bacc("TRN2", target_bir_lowering=False, debug=False, num_devices=C)

    xs_in = nc.dram_tensor("xs_in", [NPC, DIN], dt.float32, kind="ExternalInput")
    w_in = [nc.dram_tensor(f"w{i}_in", [DIN if i == 0 else DHID, FO[i]],
                           dt.float32, kind="ExternalInput") for i in range(5)]
    b_in = [nc.dram_tensor(f"b{i}_in", [128, FO[i]], dt.float32,
                           kind="ExternalInput") for i in range(5)]
    seg_in = nc.dram_tensor("seg_in", [128, nchunk, 128], dt.float16,
                            kind="ExternalInput")
    gidx_in = nc.dram_tensor("gidx_in", [128, nchunk * 8], dt.int16,
                             kind="ExternalInput")
    pool_in = nc.dram_tensor("pool_in", [128, NTILE, G], dt.float16,
                             kind="ExternalInput")
    id_in = nc.dram_tensor("id_in", [128, 128], dt.float16, kind="ExternalInput")
    out = nc.dram_tensor("out", [G, DOUT], dt.float32, kind="ExternalOutput")

    hw_sh = nc.dram_tensor("hw_sh", [NROWS, DHID], dt.float16, addr_space="Shared")
    hw_sh5 = nc.dram_tensor("hw_sh5", [NROWS, DOUT], dt.float16, addr_space="Shared")
    pool_sh = nc.dram_tensor("pool_sh", [G, DOUT], dt.float32, addr_space="Shared")
    bounce = nc.dram_tensor("bounce", [NPAD, DHID], dt.float16)
    bounce5 = nc.dram_tensor("bounce5", [NPAD, DOUT], dt.float16)
    pool_bounce = nc.dram_tensor("pool_bounce", [G, DOUT], dt.float32)

    tsz = [128] * (NTILE - 1) + [NPC - 128 * (NTILE - 1)]

    with tile.TileContext(nc) as tc:
        with (
            tc.tile_pool(name="const", bufs=1) as cp,
            tc.tile_pool(name="work", bufs=2) as wp,
            tc.tile_pool(name="msgp", bufs=3) as mp,
            tc.tile_pool(name="gemm_ps", bufs=2, space="PSUM") as gps,
            tc.tile_pool(name="agg_ps", bufs=2, space="PSUM") as aps,
            tc.tile_pool(name="tp_ps", bufs=2, space="PSUM") as tps,
            tc.tile_pool(name="pool_ps", bufs=1, space="PSUM") as pps,
        ):
            # ---- resident tensors ----
            seg_sb = cp.tile([128, nchunk, 128], dt.float16)
            nc.sync.dma_start(out=seg_sb[:, :, :], in_=seg_in[:, :, :])
            gidx_sb = cp.tile([128, nchunk * 8], dt.int16)
            nc.sync.dma_start(out=gidx_sb[:, :], in_=gidx_in[:, :])
            pool_sb = cp.tile([128, NTILE, G], dt.float16)
            nc.sync.dma_start(out=pool_sb[:, :, :], in_=pool_in[:, :, :])
            id16 = cp.tile([128, 128], dt.float16)
            nc.sync.dma_start(out=id16[:, :], in_=id_in[:, :])
            breps = cp.tile([128, 4, DHID], dt.float32)
            for l in range(4):
                nc.sync.dma_start(out=breps[:, l, :], in_=b_in[l][:, :])
            brep5 = cp.tile([128, DOUT], dt.float32)
            nc.sync.dma_start(out=brep5[:, :], in_=b_in[4][:, :])

            # weights -> fp16 tiles. slots: L1 -> w16[:,0,:]; L2..L4 -> 1+4(l-1)+j
            w16 = cp.tile([128, 13, DHID], dt.float16)
            w516 = cp.tile([128, 4, DOUT], dt.float16)
            for l in range(5):
                for j in range(FIT[l]):
                    wstage = wp.tile([128, FO[l]], dt.float32, tag="wstage")
                    nc.sync.dma_start(
                        out=wstage[:, :], in_=w_in[l][j * 128:(j + 1) * 128, :])
                    if l < 4:
                        nc.vector.tensor_copy(w16[:, (0 if l == 0 else 1 + 4 * (l - 1)) + j, :], wstage[:, :])
                    else:
                        nc.vector.tensor_copy(w516[:, j, :], wstage[:, :])

            # hT: transposed activations [128fi, tile, fi_tile, 128n]
            hT = cp.tile([128, NTILE, 4, 128], dt.float16)
            h_out = cp.tile([128, NTILE, DOUT], dt.float16)

            # x slice -> hT[:, t, 0, :]
            for t in range(NTILE):
                xstage = wp.tile([128, 128], dt.float16, tag="xstage")
                xraw = wp.tile([128, 128], dt.float32, tag="xraw")
                if tsz[t] < 128:
                    nc.vector.memset(xraw[:, :], 0.0)
                nc.sync.dma_start(
                    out=xraw[:tsz[t], :],
                    in_=xs_in[t * 128: t * 128 + tsz[t], :])
                nc.vector.tensor_copy(xstage[:, :], xraw[:, :])
                pt0 = tps.tile([128, DHID], dt.float16, tag="pt")
                nc.tensor.transpose(pt0[:, :128], xstage[:, :], id16[:, :])
                nc.vector.tensor_copy(hT[:, t, 0, :], pt0[:, :128])

            # ---- layers ----
            for l in range(5):
                fo = FO[l]
                fit = FIT[l]
                gsh = hw_sh if l < 4 else hw_sh5
                gbounce = bounce if l < 4 else bounce5

                # GEMM h @ W -> hw16 staging (node-major fp16)
                hw16 = wp.tile([128, NTILE, fo], dt.float16, tag="hw16")
                for t in range(NTILE):
                    pg = gps.tile([128, fo], dt.float32, tag="pg")
                    for j in range(fit):
                        wslot = (w16[:, (0 if l == 0 else 1 + 4 * (l - 1)) + j, :fo]
                                 if l < 4 else w516[:, j, :])
                        nc.tensor.matmul(
                            pg[:, :], hT[:, t, j, :], wslot,
                            start=(j == 0), stop=(j == fit - 1))
                    nc.vector.tensor_copy(hw16[:, t, :], pg[:, :])

                nc.sync.dma_start(
                    out=gbounce.ap().rearrange("(t p) f -> p t f", p=128),
                    in_=hw16[:, :, :])
                nc.gpsimd.collective_compute(
                    "AllGather", mybir.AluOpType.bypass,
                    replica_groups=[list(range(C))],
                    ins=[gbounce.ap().opt()],
                    outs=[gsh.ap().opt()])

                # aggregate per dst tile; gathers split into ring-safe batches
                safe = (t_pad + 1) // 2
                for t in range(NTILE):
                    pa = aps.tile([128, fo], dt.float32, tag="pa")
                    for b in range((t_pad + safe - 1) // safe):
                        k0 = b * safe
                        k1 = min(t_pad, k0 + safe)
                        msgs = mp.tile([128, safe, fo], dt.float16, tag="msgs")
                        nc.gpsimd.dma_gather(
                            out_ap=msgs[:, :k1 - k0, :],
                            in_ap=gsh[:, :],
                            idxs_ap=gidx_sb[:, (t * t_pad + k0) * 8:
                                            (t * t_pad + k1) * 8],
                            num_idxs=(k1 - k0) * 128,
                            num_idxs_reg=(k1 - k0) * 128,
                            elem_size=fo,
                            single_packet=False)
                        for k in range(k0, k1):
                            nc.tensor.matmul(
                                pa[:, :], seg_sb[:, t * t_pad + k, :],
                                msgs[:, k - k0, :],
                                start=(k == 0), stop=(k == t_pad - 1))
                    hsum = wp.tile([128, fo], dt.float32, tag="hsum")
                    nc.vector.tensor_tensor(
                        hsum[:, :], pa[:, :],
                        breps[:, l, :fo] if l < 4 else brep5[:, :],
                        mybir.AluOpType.add)
                    if l < 4:
                        hnm = wp.tile([128, fo], dt.float16, tag="hnm")
                        nc.vector.tensor_scalar_max(hnm[:, :], hsum[:, :], 0.0)
                        pt = tps.tile([128, fo], dt.float16, tag="pt")
                        for j in range(fo // 128):
                            nc.tensor.transpose(
                                pt[:, j * 128:(j + 1) * 128],
                                hnm[:, j * 128:(j + 1) * 128], id16[:, :])
                        nc.vector.tensor_copy(
                            hT[:, t, :, :].rearrange("p a b -> p (a b)"),
                            pt[:, :])
                    else:
                        nc.vector.tensor_scalar_max(
                            h_out[:, t, :], hsum[:, :], 0.0)

            # ---- mean pool ----
            pp = pps.tile([64, DOUT], dt.float32)
            for t in range(NTILE):
                nc.tensor.matmul(
                    pp[:, :], pool_sb[:, t, :64], h_out[:, t, :],
                    start=(t == 0), stop=(t == NTILE - 1))
            pres = wp.tile([64, DOUT], dt.float32, tag="pres")
            nc.vector.tensor_copy(pres[:, :], pp[:, :])
            nc.sync.dma_start(out=pool_bounce[:, :], in_=pres[:, :])
            nc.gpsimd.collective_compute(
                "AllReduce", mybir.AluOpType.add,
                replica_groups=[list(range(C))],
                ins=[pool_bounce.ap().opt()],
                outs=[pool_sh.ap().opt()])
            ores = wp.tile([64, DOUT], dt.float32, tag="ores")
            nc.sync.dma_start(out=ores[:, :], in_=pool_sh[:, :])
            nc.sync.dma_start(out=out[:, :], in_=ores[:, :])

    nc.compile()
    return nc


_CACHE = {}


def _get_program(t_pad, nchunk):
    key = (t_pad, nchunk)
    if key not in _CACHE:
        _CACHE[key] = _build(t_pad, nchunk)
    return _CACHE[key]


def make_in_maps(inputs):
    edge_index = np.asarray(inputs["edge_index"])
    batch = np.asarray(inputs["batch"])
    x = np.asarray(inputs["x"], dtype=np.float32)
    gidx_w, seg, pool, t_pad, nchunk = _preprocess(edge_index, batch)
    ident = np.eye(128, dtype=np.float16)
    in_maps = []
    for c in range(C):
        m = {
            "xs_in": np.ascontiguousarray(x[c * NPC:(c + 1) * NPC]),
            "seg_in": np.ascontiguousarray(seg[c]),
            "gidx_in": gidx_w[c],
            "pool_in": np.ascontiguousarray(pool[c]),
            "id_in": ident,
        }
        for i in range(5):
            w = np.asarray(inputs[f"W{i + 1}"], dtype=np.float32)
            b = np.asarray(inputs[f"b{i + 1}"], dtype=np.float32)
            m[f"w{i}_in"] = w
            m[f"b{i}_in"] = np.ascontiguousarray(np.tile(b[None, :], (128, 1)))
        in_maps.append(m)
    return in_maps, t_pad, nchunk


def kernel(**inputs):
    in_maps, t_pad, nchunk = make_in_maps(inputs)
    nc = _get_program(t_pad, nchunk)
    res = bass_utils.run_bass_kernel_spmd(
        nc, in_maps, core_ids=list(range(C)))
    return res.results[0]["out"].astype(np.float32)



# revision 3
# speedup vs baseline: 1.0771x; 1.0771x over previous
"""GCN encoder (5-layer GCNConv + global mean pool) on 8 Trainium2 NeuronCores.

Strategy (node sharding):
  - 10000 nodes split contiguously across 8 cores (1250/core, padded to 1280).
  - Each layer: per-core GEMM (h @ W, fp16 operands, fp32 PSUM) ->
    AllGather of the fp16 hW slices into pair-shared DRAM ->
    dma_gather of per-edge source rows (dst-sorted, chunked 128 edges) ->
    segment-sum as one-hot matmul per chunk (Seg[128e,128d].T @ msgs[128e,fo]
    accumulated in PSUM per 128-dst tile; GCN norm folded into Seg values) ->
    bias + relu (DVE) -> PE transpose to keep h^T for the next GEMM.
  - Mean-pool as matmul with 1/count one-hot, AllReduce over cores.

The graph structure (edge sort, one-hot Seg with norm, gather indices,
pool matrix) is preprocessed on host; all FLOPs on x/W run on device.
"""
import sys

import numpy as np

sys.path.insert(0, "/opt/trn_rl_repo")

import concourse.bacc as bacc
import concourse.bass as bass  # noqa: F401
import concourse.mybir as mybir
import concourse.tile as tile
from concourse import bass_utils

dt = mybir.dt

N = 10000
E = 150000
G = 64
DIN = 128
DHID = 512
DOUT = 128
C = 8
NPC = N // C          # 1250 nodes per core
NTILE = 10            # ceil(1250/128)
NPAD = NTILE * 128    # 1280 padded rows per core
NROWS = C * NPAD      # 10240 rows in the allgathered table
FO = [DHID, DHID, DHID, DHID, DOUT]
FIT = [1, 4, 4, 4, 4]  # fi tiles per layer (fi=128 for L1, 512 for L2-5)


def _preprocess(edge_index, batch):
    """Build per-core gather indices, one-hot Seg blocks, and pool matrix."""
    src = np.concatenate([edge_index[0], np.arange(N, dtype=np.int64)])
    dst = np.concatenate([edge_index[1], np.arange(N, dtype=np.int64)])
    deg = np.bincount(dst, minlength=N).astype(np.float64)
    dinv = np.where(deg > 0, 1.0 / np.sqrt(deg), 0.0)
    norm = (dinv[src] * dinv[dst]).astype(np.float32)

    core = dst // NPC
    t_of = (dst % NPC) // 128
    dloc = (dst % NPC) % 128

    # edge counts per (core, tile) -> uniform chunk budget T_pad
    cnt = np.zeros((C, NTILE), np.int64)
    np.add.at(cnt, (core, t_of), 1)
    t_pad = int(np.ceil(cnt.max() / 128))
    nchunk = NTILE * t_pad

    # slot of each edge inside its (core, tile) bucket
    order = np.lexsort((dst, t_of, core))
    s_src, s_core, s_t, s_dloc, s_norm = (
        src[order], core[order], t_of[order], dloc[order], norm[order])
    # position within bucket
    bucket = s_core * NTILE + s_t
    start = np.zeros(C * NTILE, np.int64)
    start[1:] = np.cumsum(np.bincount(bucket, minlength=C * NTILE))[:-1]
    pos = np.arange(len(s_src)) - start[bucket]

    chunk = s_t * t_pad + pos // 128     # chunk id within core
    erow = pos % 128                     # row within chunk

    # padded-row index of each source node in the allgathered table
    srow = (s_src // NPC) * NPAD + (s_src % NPC)

    gidx = np.zeros((C, nchunk * 128), np.int16)
    seg = np.zeros((C, 128, nchunk, 128), np.float16)
    gidx[s_core, chunk * 128 + erow] = srow.astype(np.int16)
    seg[s_core, erow, chunk, s_dloc] = s_norm.astype(np.float16)

    # idx wrap: logical idx i -> partition i%16, column i//16; replicate x8
    gidx_w = np.ascontiguousarray(
        np.tile(gidx.reshape(C, -1, 16).transpose(0, 2, 1), (1, 8, 1)))

    # pool matrix [C, 128, NTILE, G]: 1/count at (node row, graph)
    gcnt = np.bincount(batch, minlength=G).astype(np.float64)
    inv = (1.0 / np.maximum(gcnt, 1.0))
    pool = np.zeros((C, 128, NTILE, G), np.float16)
    nodes = np.arange(N)
    pc, pr = nodes // NPC, nodes % NPC
    pool[pc, pr % 128, pr // 128, batch] = inv[batch].astype(np.float16)

    return gidx_w, seg, pool, t_pad, nchunk


def _build(t_pad, nchunk):
    nc = bacc.Bacc("TRN2", target_bir_lowering=False, debug=False, num_devices=C,
                   num_swdge_queues=4, dynamic_dma_scratch_size=32768)

    xs_in = nc.dram_tensor("xs_in", [NPC, DIN], dt.float32, kind="ExternalInput")
    w_in = [nc.dram_tensor(f"w{i}_in", [DIN if i == 0 else DHID, FO[i]],
                           dt.float32, kind="ExternalInput") for i in range(5)]
    b_in = [nc.dram_tensor(f"b{i}_in", [128, FO[i]], dt.float32,
                           kind="ExternalInput") for i in range(5)]
    seg_in = nc.dram_tensor("seg_in", [128, nchunk, 128], dt.float16,
                            kind="ExternalInput")
    gidx_in = nc.dram_tensor("gidx_in", [128, nchunk * 8], dt.int16,
                             kind="ExternalInput")
    pool_in = nc.dram_tensor("pool_in", [128, NTILE, G], dt.float16,
                             kind="ExternalInput")
    id_in = nc.dram_tensor("id_in", [128, 128], dt.float16, kind="ExternalInput")
    out = nc.dram_tensor("out", [G, DOUT], dt.float32, kind="ExternalOutput")

    hw_sh = nc.dram_tensor("hw_sh", [NROWS, DHID], dt.float16, addr_space="Shared")
    hw_sh5 = nc.dram_tensor("hw_sh5", [NROWS, DOUT], dt.float16, addr_space="Shared")
    pool_sh = nc.dram_tensor("pool_sh", [G, DOUT], dt.float32, addr_space="Shared")
    bounce = nc.dram_tensor("bounce", [NPAD, DHID], dt.float16)
    bounce5 = nc.dram_tensor("bounce5", [NPAD, DOUT], dt.float16)
    pool_bounce = nc.dram_tensor("pool_bounce", [G, DOUT], dt.float32)

    tsz = [128] * (NTILE - 1) + [NPC - 128 * (NTILE - 1)]

    with tile.TileContext(nc) as tc:
        with (
            tc.tile_pool(name="const", bufs=1) as cp,
            tc.tile_pool(name="work", bufs=2) as wp,
            tc.tile_pool(name="msgp", bufs=3) as mp,
            tc.tile_pool(name="gemm_ps", bufs=2, space="PSUM") as gps,
            tc.tile_pool(name="agg_ps", bufs=2, space="PSUM") as aps,
            tc.tile_pool(name="tp_ps", bufs=2, space="PSUM") as tps,
            tc.tile_pool(name="pool_ps", bufs=1, space="PSUM") as pps,
        ):
            # ---- resident tensors ----
            seg_sb = cp.tile([128, nchunk, 128], dt.float16)
            nc.sync.dma_start(out=seg_sb[:, :, :], in_=seg_in[:, :, :])
            gidx_sb = cp.tile([128, nchunk * 8], dt.int16)
            nc.sync.dma_start(out=gidx_sb[:, :], in_=gidx_in[:, :])
            pool_sb = cp.tile([128, NTILE, G], dt.float16)
            nc.sync.dma_start(out=pool_sb[:, :, :], in_=pool_in[:, :, :])
            id16 = cp.tile([128, 128], dt.float16)
            nc.sync.dma_start(out=id16[:, :], in_=id_in[:, :])
            breps = cp.tile([128, 4, DHID], dt.float32)
            for l in range(4):
                nc.sync.dma_start(out=breps[:, l, :], in_=b_in[l][:, :])
            brep5 = cp.tile([128, DOUT], dt.float32)
            nc.sync.dma_start(out=brep5[:, :], in_=b_in[4][:, :])

            # weights -> fp16 tiles. slots: L1 -> w16[:,0,:]; L2..L4 -> 1+4(l-1)+j
            w16 = cp.tile([128, 13, DHID], dt.float16)
            w516 = cp.tile([128, 4, DOUT], dt.float16)
            for l in range(5):
                for j in range(FIT[l]):
                    wstage = wp.tile([128, FO[l]], dt.float32, tag="wstage")
                    nc.sync.dma_start(
                        out=wstage[:, :], in_=w_in[l][j * 128:(j + 1) * 128, :])
                    if l < 4:
                        nc.vector.tensor_copy(w16[:, (0 if l == 0 else 1 + 4 * (l - 1)) + j, :], wstage[:, :])
                    else:
                        nc.vector.tensor_copy(w516[:, j, :], wstage[:, :])

            # hT: transposed activations [128fi, tile, fi_tile, 128n]
            hT = cp.tile([128, NTILE, 4, 128], dt.float16)
            h_out = cp.tile([128, NTILE, DOUT], dt.float16)

            # x slice -> hT[:, t, 0, :]
            for t in range(NTILE):
                xstage = wp.tile([128, 128], dt.float16, tag="xstage")
                xraw = wp.tile([128, 128], dt.float32, tag="xraw")
                if tsz[t] < 128:
                    nc.vector.memset(xraw[:, :], 0.0)
                nc.sync.dma_start(
                    out=xraw[:tsz[t], :],
                    in_=xs_in[t * 128: t * 128 + tsz[t], :])
                nc.vector.tensor_copy(xstage[:, :], xraw[:, :])
                pt0 = tps.tile([128, DHID], dt.float16, tag="pt")
                nc.tensor.transpose(pt0[:, :128], xstage[:, :], id16[:, :])
                nc.vector.tensor_copy(hT[:, t, 0, :], pt0[:, :128])

            # ---- layers ----
            for l in range(5):
                fo = FO[l]
                fit = FIT[l]
                gsh = hw_sh if l < 4 else hw_sh5
                gbounce = bounce if l < 4 else bounce5

                # GEMM h @ W -> hw16 staging (node-major fp16)
                hw16 = wp.tile([128, NTILE, fo], dt.float16, tag="hw16")
                for t in range(NTILE):
                    pg = gps.tile([128, fo], dt.float32, tag="pg")
                    for j in range(fit):
                        wslot = (w16[:, (0 if l == 0 else 1 + 4 * (l - 1)) + j, :fo]
                                 if l < 4 else w516[:, j, :])
                        nc.tensor.matmul(
                            pg[:, :], hT[:, t, j, :], wslot,
                            start=(j == 0), stop=(j == fit - 1))
                    nc.vector.tensor_copy(hw16[:, t, :], pg[:, :])

                nc.sync.dma_start(
                    out=gbounce.ap().rearrange("(t p) f -> p t f", p=128),
                    in_=hw16[:, :, :])
                nc.gpsimd.collective_compute(
                    "AllGather", mybir.AluOpType.bypass,
                    replica_groups=[list(range(C))],
                    ins=[gbounce.ap().opt()],
                    outs=[gsh.ap().opt()])

                # aggregate per dst tile; gathers split into ring-safe batches
                safe = (t_pad + 1) // 2
                for t in range(NTILE):
                    pa = aps.tile([128, fo], dt.float32, tag="pa")
                    for b in range((t_pad + safe - 1) // safe):
                        k0 = b * safe
                        k1 = min(t_pad, k0 + safe)
                        msgs = mp.tile([128, safe, fo], dt.float16, tag="msgs")
                        nc.gpsimd.dma_gather(
                            out_ap=msgs[:, :k1 - k0, :],
                            in_ap=gsh[:, :],
                            idxs_ap=gidx_sb[:, (t * t_pad + k0) * 8:
                                            (t * t_pad + k1) * 8],
                            num_idxs=(k1 - k0) * 128,
                            num_idxs_reg=(k1 - k0) * 128,
                            elem_size=fo,
                            single_packet=False,
                            queue_num=(t * 2 + b) % 4)
                        for k in range(k0, k1):
                            nc.tensor.matmul(
                                pa[:, :], seg_sb[:, t * t_pad + k, :],
                                msgs[:, k - k0, :],
                                start=(k == 0), stop=(k == t_pad - 1))
                    hsum = wp.tile([128, fo], dt.float32, tag="hsum")
                    nc.vector.tensor_tensor(
                        hsum[:, :], pa[:, :],
                        breps[:, l, :fo] if l < 4 else brep5[:, :],
                        mybir.AluOpType.add)
                    if l < 4:
                        hnm = wp.tile([128, fo], dt.float16, tag="hnm")
                        nc.vector.tensor_scalar_max(hnm[:, :], hsum[:, :], 0.0)
                        pt = tps.tile([128, fo], dt.float16, tag="pt")
                        for j in range(fo // 128):
                            nc.tensor.transpose(
                                pt[:, j * 128:(j + 1) * 128],
                                hnm[:, j * 128:(j + 1) * 128], id16[:, :])
                        nc.vector.tensor_copy(
                            hT[:, t, :, :].rearrange("p a b -> p (a b)"),
                            pt[:, :])
                    else:
                        nc.vector.tensor_scalar_max(
                            h_out[:, t, :], hsum[:, :], 0.0)

            # ---- mean pool ----
            pp = pps.tile([64, DOUT], dt.float32)
            for t in range(NTILE):
                nc.tensor.matmul(
                    pp[:, :], pool_sb[:, t, :64], h_out[:, t, :],
                    start=(t == 0), stop=(t == NTILE - 1))
            pres = wp.tile([64, DOUT], dt.float32, tag="pres")
            nc.vector.tensor_copy(pres[:, :], pp[:, :])
            nc.sync.dma_start(out=pool_bounce[:, :], in_=pres[:, :])
            nc.gpsimd.collective_compute(
                "AllReduce", mybir.AluOpType.add,
                replica_groups=[list(range(C))],
                ins=[pool_bounce.ap().opt()],
                outs=[pool_sh.ap().opt()])
            ores = wp.tile([64, DOUT], dt.float32, tag="ores")
            nc.sync.dma_start(out=ores[:, :], in_=pool_sh[:, :])
            nc.sync.dma_start(out=out[:, :], in_=ores[:, :])

    nc.compile()
    return nc


_CACHE = {}


def _get_program(t_pad, nchunk):
    key = (t_pad, nchunk)
    if key not in _CACHE:
        _CACHE[key] = _build(t_pad, nchunk)
    return _CACHE[key]


def make_in_maps(inputs):
    edge_index = np.asarray(inputs["edge_index"])
    batch = np.asarray(inputs["batch"])
    x = np.asarray(inputs["x"], dtype=np.float32)
    gidx_w, seg, pool, t_pad, nchunk = _preprocess(edge_index, batch)
    ident = np.eye(128, dtype=np.float16)
    in_maps = []
    for c in range(C):
        m = {
            "xs_in": np.ascontiguousarray(x[c * NPC:(c + 1) * NPC]),
            "seg_in": np.ascontiguousarray(seg[c]),
            "gidx_in": gidx_w[c],
            "pool_in": np.ascontiguousarray(pool[c]),
            "id_in": ident,
        }
        for i in range(5):
            w = np.asarray(inputs[f"W{i + 1}"], dtype=np.float32)
            b = np.asarray(inputs[f"b{i + 1}"], dtype=np.float32)
            m[f"w{i}_in"] = w
            m[f"b{i}_in"] = np.ascontiguousarray(np.tile(b[None, :], (128, 1)))
        in_maps.append(m)
    return in_maps, t_pad, nchunk


def kernel(**inputs):
    in_maps, t_pad, nchunk = make_in_maps(inputs)
    nc = _get_program(t_pad, nchunk)
    res = bass_utils.run_bass_kernel_spmd(
        nc, in_maps, core_ids=list(range(C)))
    return res.results[0]["out"].astype(np.float32)



# revision 12
# speedup vs baseline: 1.8704x; 1.7364x over previous
"""GCN encoder (5-layer GCNConv + global mean pool) on 8 Trainium2 NeuronCores.

v2 strategy (node sharding, tile-pipelined):
  - 10000 nodes split contiguously across 8 cores (1250/core, padded to 1280).
  - Conv1 is refactored as relu((A_hat @ x) W1 + b1): the per-edge gather runs
    on the 128-wide x table (256B rows, no collective needed - x is an input).
  - Conv i (i>=2): table_i = h_{i-1} @ W_i computed tile-by-tile in the
    PREVIOUS conv's tile loop (cross-layer pipelining), AllGathered to every
    core, then per-edge dma_gather (dst-sorted chunks, 4 SWDGE queues rotated)
    feeds one-hot segment-sum matmuls on the PE (GCN norm folded into Seg).
  - Bias is a K=1 rank-1 matmul into the same PSUM accumulation group;
    relu + fp32->fp16 cast runs on the Scalar (ACT) engine straight out of
    PSUM; activation transposes stay on the PE.
  - Trailing padded slots carry index -1: SWDGE descriptor generation and the
    DMA drain skip them (the one-hot Seg weights for those slots are 0).
  - Mean-pool as matmul with 1/count one-hot, AllReduce over cores.
"""
import sys

import numpy as np

sys.path.insert(0, "/opt/trn_rl_repo")

import concourse.bacc as bacc
import concourse.bass as bass  # noqa: F401
import concourse.mybir as mybir
import concourse.tile as tile
from concourse import bass_utils

dt = mybir.dt
AF = mybir.ActivationFunctionType

N = 10000
E = 150000
G = 64
DIN = 128
DHID = 512
DOUT = 128
C = 8
NPC = N // C          # 1250 nodes per core
NTILE = 10            # ceil(1250/128)
NPAD = NTILE * 128    # 1280 padded rows per core
NROWS = C * NPAD      # 10240 rows in the gathered tables
WG = [DIN, DHID, DHID, DHID, DOUT]   # gather width for conv i
FO = [DHID, DHID, DHID, DOUT]        # GEMM output width (W2..W5)
FIT = [4, 4, 4, 4]                   # fi tiles for W2..W5 (fi=512)


def _preprocess(edge_index, batch, trim=True):
    """Per-core gather indices (trailing -1 padded), one-hot Seg, pool mat."""
    src = np.concatenate([edge_index[0], np.arange(N, dtype=np.int64)])
    dst = np.concatenate([edge_index[1], np.arange(N, dtype=np.int64)])
    deg = np.bincount(dst, minlength=N).astype(np.float64)
    dinv = np.where(deg > 0, 1.0 / np.sqrt(deg), 0.0)
    norm = (dinv[src] * dinv[dst]).astype(np.float32)

    core = dst // NPC
    t_of = (dst % NPC) // 128
    dloc = (dst % NPC) % 128

    cnt = np.zeros((C, NTILE), np.int64)
    np.add.at(cnt, (core, t_of), 1)
    t_pad = int(np.ceil(cnt.max() / 128))
    nchunk = NTILE * t_pad

    order = np.lexsort((dst, t_of, core))
    s_src, s_core, s_t, s_dloc, s_norm = (
        src[order], core[order], t_of[order], dloc[order], norm[order])
    bucket = s_core * NTILE + s_t
    start = np.zeros(C * NTILE, np.int64)
    start[1:] = np.cumsum(np.bincount(bucket, minlength=C * NTILE))[:-1]
    pos = np.arange(len(s_src)) - start[bucket]

    chunk = s_t * t_pad + pos // 128
    erow = pos % 128

    srow = (s_src // NPC) * NPAD + (s_src % NPC)

    # -1 marks padded slots; only trailing ones per gather batch are skipped
    # by HW, interior ones would fault, so flip non-trailing back to 0.
    fill = -1 if trim else 0
    gidx = np.full((C, nchunk * 128), fill, np.int16)
    seg = np.zeros((C, 128, nchunk, 128), np.float16)
    gidx[s_core, chunk * 128 + erow] = srow.astype(np.int16)
    seg[s_core, erow, chunk, s_dloc] = s_norm.astype(np.float16)

    # batches: per tile, chunks [0, safe) and [safe, t_pad). Trailing -1s
    # within each batch's flat idx list are skippable; others -> 0.
    safe = (t_pad + 1) // 2
    g3 = gidx.reshape(C, NTILE, t_pad * 128)
    for (b0, b1) in (((0, safe), (safe, t_pad)) if trim else ()):
        blk = g3[:, :, b0 * 128:b1 * 128]
        flat = blk.reshape(C * NTILE, -1)
        for r in range(flat.shape[0]):
            neg = np.flatnonzero(flat[r] >= 0)
            last = neg[-1] + 1 if len(neg) else 0
            flat[r, :last][flat[r, :last] < 0] = 0

    # idx wrap: slot i -> partition i%16, column i//16; replicate x8
    gidx_w = np.ascontiguousarray(
        np.tile(gidx.reshape(C, -1, 16).transpose(0, 2, 1), (1, 8, 1)))

    gcnt = np.bincount(batch, minlength=G).astype(np.float64)
    inv = (1.0 / np.maximum(gcnt, 1.0))
    pool = np.zeros((C, 128, NTILE, G), np.float16)
    nodes = np.arange(N)
    pc, pr = nodes // NPC, nodes % NPC
    pool[pc, pr % 128, pr // 128, batch] = inv[batch].astype(np.float16)

    safe2 = (t_pad + 1) // 2
    nval = np.zeros((C, NTILE, 2), np.int32)
    for b, (b0, b1) in enumerate(((0, safe2), (safe2, t_pad))):
        nval[:, :, b] = np.clip(cnt - b0 * 128, 0, (b1 - b0) * 128)
    return gidx_w, seg, pool, t_pad, nchunk, nval


def _build(t_pad, nchunk):
    nc = bacc.Bacc("TRN2", target_bir_lowering=False, debug=False,
                   num_devices=C, num_swdge_queues=4,
                   dynamic_dma_scratch_size=32768)

    xt_in = nc.dram_tensor("xt_in", [NROWS, DIN], dt.float16,
                           kind="ExternalInput")
    w_in = [nc.dram_tensor(f"w{i}_in", [DIN if i == 0 else DHID, FO[i - 1] if i else DHID],
                           dt.float32, kind="ExternalInput") for i in range(5)]
    b_in = [nc.dram_tensor(f"b{i}_in", [1, DHID if i < 4 else DOUT], dt.float32,
                           kind="ExternalInput") for i in range(5)]
    seg_in = nc.dram_tensor("seg_in", [128, nchunk, 128], dt.float16,
                            kind="ExternalInput")
    gidx_in = nc.dram_tensor("gidx_in", [128, nchunk * 8], dt.int16,
                             kind="ExternalInput")
    pool_in = nc.dram_tensor("pool_in", [128, NTILE, G], dt.float16,
                             kind="ExternalInput")
    id_in = nc.dram_tensor("id_in", [128, 128], dt.float16, kind="ExternalInput")
    ones_in = nc.dram_tensor("ones_in", [1, 128], dt.float16,
                             kind="ExternalInput")
    nval_in = nc.dram_tensor("nval_in", [1, NTILE * 2], dt.int32,
                             kind="ExternalInput")
    out = nc.dram_tensor("out", [G, DOUT], dt.float32, kind="ExternalOutput")

    hw_sh = nc.dram_tensor("hw_sh", [NROWS, DHID], dt.float16, addr_space="Shared")
    hw_sh5 = nc.dram_tensor("hw_sh5", [NROWS, DOUT], dt.float16, addr_space="Shared")
    pool_sh = nc.dram_tensor("pool_sh", [G, DOUT], dt.float32, addr_space="Shared")
    bounce = nc.dram_tensor("bounce", [NPAD, DHID], dt.float16)
    bounce5 = nc.dram_tensor("bounce5", [NPAD, DOUT], dt.float16)
    pool_bounce = nc.dram_tensor("pool_bounce", [G, DOUT], dt.float32)

    safe = (t_pad + 1) // 2
    batches = [(0, safe), (safe, t_pad)]

    with tile.TileContext(nc) as tc:
        with (
            tc.tile_pool(name="const", bufs=1) as cp,
            tc.tile_pool(name="work", bufs=2) as wp,
            tc.tile_pool(name="msgp", bufs=4) as mp,
            tc.tile_pool(name="gemm_ps", bufs=2, space="PSUM") as gps,
            tc.tile_pool(name="agg_ps", bufs=2, space="PSUM") as aps,
            tc.tile_pool(name="tp_ps", bufs=2, space="PSUM") as tps,
            tc.tile_pool(name="pool_ps", bufs=1, space="PSUM") as pps,
        ):
            # ---- resident tensors ----
            seg_sb = cp.tile([128, nchunk, 128], dt.float16)
            nc.sync.dma_start(out=seg_sb[:, :, :], in_=seg_in[:, :, :])
            gidx_sb = cp.tile([128, nchunk * 8], dt.int16)
            nc.sync.dma_start(out=gidx_sb[:, :], in_=gidx_in[:, :])
            pool_sb = cp.tile([128, NTILE, G], dt.float16)
            nc.sync.dma_start(out=pool_sb[:, :, :], in_=pool_in[:, :, :])
            id16 = cp.tile([128, 128], dt.float16)
            nc.sync.dma_start(out=id16[:, :], in_=id_in[:, :])
            ones1 = cp.tile([1, 128], dt.float16)
            nc.sync.dma_start(out=ones1[:, :], in_=ones_in[:, :])
            nval_sb = cp.tile([1, NTILE * 2], dt.int32)
            nc.sync.dma_start(out=nval_sb[:, :], in_=nval_in[:, :])
            nreg = nc.alloc_register(mybir.EngineType.Pool, "nreg")
            brow = cp.tile([1, 5, DHID], dt.float16)
            for i in range(5):
                braw = wp.tile([1, DHID if i < 4 else DOUT], dt.float32, tag="braw")
                nc.sync.dma_start(out=braw[:, :], in_=b_in[i][:, :])
                nc.vector.tensor_copy(
                    brow[:, i, :DHID if i < 4 else DOUT], braw[:, :])

            # weights -> fp16. w16 slots: W1 -> 0; W2..W4 -> 1+4(i-2)+j
            w16 = cp.tile([128, 13, DHID], dt.float16)
            w516 = cp.tile([128, 4, DOUT], dt.float16)
            for i in range(5):
                nj = 1 if i == 0 else 4
                for j in range(nj):
                    wstage = wp.tile([128, FO[i - 1] if i else DHID],
                                     dt.float32, tag="wstage")
                    nc.sync.dma_start(
                        out=wstage[:, :], in_=w_in[i][j * 128:(j + 1) * 128, :])
                    if i < 4:
                        nc.vector.tensor_copy(
                            w16[:, (0 if i == 0 else 1 + 4 * (i - 1)) + j, :],
                            wstage[:, :])
                    else:
                        nc.vector.tensor_copy(w516[:, j, :], wstage[:, :])

            # prime msgs pool bufs so skipped (trimmed) slots hold finite
            # values: garbage x 0 seg weight must be 0, not NaN.
            for w in range(4):
                for b, (k0, k1) in enumerate(batches):
                    mprime = mp.tile([128, k1 - k0, DHID], dt.float16,
                                     tag=f"msgs{b}", name=f"mprime{w}_{b}")
                    nc.vector.memset(
                        mprime[:, :, :].rearrange("p a b -> p (a b)"), 0.0)

            hT = cp.tile([128, NTILE, 4, 128], dt.float16)
            h_out = cp.tile([128, NTILE, DOUT], dt.float16)
            hw16 = cp.tile([128, NTILE, DHID], dt.float16)
            hw516 = cp.tile([128, NTILE, DOUT], dt.float16)

            def gathers(i, t):
                """Issue the per-tile gather batches for conv i."""
                wg = WG[i]
                gtab = (xt_in if i == 0 else (hw_sh if i < 4 else hw_sh5))
                ms = []
                for b, (k0, k1) in enumerate(batches):
                    msgs = mp.tile([128, k1 - k0, wg], dt.float16,
                                   tag=f"msgs{b}")
                    nc.gpsimd.reg_load(
                        nreg, nval_sb[0:1, t * 2 + b:t * 2 + b + 1])
                    nc.gpsimd.dma_gather(
                        out_ap=msgs[:, :, :],
                        in_ap=gtab[:, :],
                        idxs_ap=gidx_sb[:, (t * t_pad + k0) * 8:
                                        (t * t_pad + k1) * 8],
                        num_idxs=(k1 - k0) * 128,
                        num_idxs_reg=nreg,
                        elem_size=wg,
                        single_packet=False,
                        queue_num=(t * 2 + b) % 4)
                    ms.append(msgs)
                return ms

            def agg(i, t, ms, with_bias):
                """Chunk matmuls (+ optional bias) into one PSUM group."""
                wg = WG[i]
                pa = aps.tile([128, wg], dt.float32, tag="pa")
                for b, (k0, k1) in enumerate(batches):
                    for k in range(k0, k1):
                        nc.tensor.matmul(
                            pa[:, :], seg_sb[:, t * t_pad + k, :],
                            ms[b][:, k - k0, :],
                            start=(k == 0),
                            stop=(not with_bias and k == t_pad - 1))
                if with_bias:
                    nc.tensor.matmul(pa[:, :], ones1[:, :], brow[:, i, :wg],
                                     start=False, stop=True)
                return pa

            def transpose_to(dst_ap, src_sb, width):
                pt = tps.tile([128, width], dt.float16, tag="pt")
                for j in range(width // 128):
                    nc.tensor.transpose(
                        pt[:, j * 128:(j + 1) * 128],
                        src_sb[:, j * 128:(j + 1) * 128], id16[:, :])
                nc.vector.tensor_copy(dst_ap, pt[:, :])

            def gemm(i, t, src_tiles, fo, fit, out_tile):
                """out_tile[:, :fo] (fp16) = src_tiles @ W_{i+1} (no bias)."""
                pg = gps.tile([128, fo], dt.float32, tag="pg")
                for j in range(fit):
                    wslot = (w16[:, 1 + 4 * (i - 1) + j, :fo] if i < 4
                             else w516[:, j, :])
                    nc.tensor.matmul(
                        pg[:, :], src_tiles[:, j, :], wslot,
                        start=(j == 0), stop=(j == fit - 1))
                nc.vector.tensor_copy(out_tile, pg[:, :])

            # ================= conv1 (aggregate x, then W1) =================
            for t in range(NTILE):
                ms = gathers(0, t)
                pa = agg(0, t, ms, with_bias=False)
                ax16 = wp.tile([128, 128], dt.float16, tag="ax16")
                nc.scalar.activation(ax16[:, :], pa[:, :], AF.Copy)
                axT = wp.tile([128, 128], dt.float16, tag="axT")
                transpose_to(axT[:, :], ax16, 128)
                # GEMM1 + b1 + relu -> h1 tile
                pg1 = gps.tile([128, DHID], dt.float32, tag="pg")
                nc.tensor.matmul(pg1[:, :], axT[:, :], w16[:, 0, :],
                                 start=True, stop=False)
                nc.tensor.matmul(pg1[:, :], ones1[:, :], brow[:, 0, :],
                                 start=False, stop=True)
                h1 = wp.tile([128, DHID], dt.float16, tag="hsb")
                nc.scalar.activation(h1[:, :], pg1[:, :], AF.Relu)
                transpose_to(hT[:, t, :, :].rearrange("p a b -> p (a b)"),
                             h1, DHID)
                # GEMM2 -> table for conv2
                gemm(1, t, hT[:, t, :, :], DHID, 4, hw16[:, t, :])
                nc.sync.dma_start(out=bounce.ap()[t * 128:(t + 1) * 128, :],
                                  in_=hw16[:, t, :])
            nc.gpsimd.collective_compute(
                "AllGather", mybir.AluOpType.bypass,
                replica_groups=[list(range(C))],
                ins=[bounce.ap().opt()], outs=[hw_sh.ap().opt()])

            # ================= conv2..conv4 =================
            for i in range(1, 4):
                for t in range(NTILE):
                    ms = gathers(i, t)
                    pa = agg(i, t, ms, with_bias=True)
                    hi = wp.tile([128, DHID], dt.float16, tag="hsb")
                    nc.scalar.activation(hi[:, :], pa[:, :], AF.Relu)
                    transpose_to(hT[:, t, :, :].rearrange("p a b -> p (a b)"),
                                 hi, DHID)
                    fo = FO[i]
                    gemm(i + 1, t, hT[:, t, :, :], fo, 4,
                         hw16[:, t, :fo] if i < 3 else hw516[:, t, :])
                    gb = bounce if i < 3 else bounce5
                    gt = hw16[:, t, :fo] if i < 3 else hw516[:, t, :]
                    nc.sync.dma_start(out=gb.ap()[t * 128:(t + 1) * 128, :fo],
                                      in_=gt)
                nc.gpsimd.collective_compute(
                    "AllGather", mybir.AluOpType.bypass,
                    replica_groups=[list(range(C))],
                    ins=[(bounce if i < 3 else bounce5).ap().opt()],
                    outs=[(hw_sh if i < 3 else hw_sh5).ap().opt()])

            # ================= conv5 + pool =================
            pp = pps.tile([64, DOUT], dt.float32)
            for t in range(NTILE):
                ms = gathers(4, t)
                pa = agg(4, t, ms, with_bias=True)
                nc.scalar.activation(h_out[:, t, :], pa[:, :], AF.Relu)
                nc.tensor.matmul(
                    pp[:, :], pool_sb[:, t, :64], h_out[:, t, :],
                    start=(t == 0), stop=(t == NTILE - 1))
            pres = wp.tile([64, DOUT], dt.float32, tag="pres")
            nc.vector.tensor_copy(pres[:, :], pp[:, :])
            nc.sync.dma_start(out=pool_bounce[:, :], in_=pres[:, :])
            nc.gpsimd.collective_compute(
                "AllReduce", mybir.AluOpType.add,
                replica_groups=[list(range(C))],
                ins=[pool_bounce.ap().opt()], outs=[pool_sh.ap().opt()])
            ores = wp.tile([64, DOUT], dt.float32, tag="ores")
            nc.sync.dma_start(out=ores[:, :], in_=pool_sh[:, :])
            nc.sync.dma_start(out=out[:, :], in_=ores[:, :])

    nc.compile()
    return nc


_CACHE = {}


def _get_program(t_pad, nchunk):
    key = (t_pad, nchunk)
    if key not in _CACHE:
        _CACHE[key] = _build(t_pad, nchunk)
    return _CACHE[key]


def make_in_maps(inputs, trim=True):
    edge_index = np.asarray(inputs["edge_index"])
    batch = np.asarray(inputs["batch"])
    x = np.asarray(inputs["x"], dtype=np.float32)
    gidx_w, seg, pool, t_pad, nchunk, nval = _preprocess(edge_index, batch, trim=trim)
    ident = np.eye(128, dtype=np.float16)
    ones = np.ones((1, 128), dtype=np.float16)
    xt = np.zeros((NROWS, DIN), np.float16)
    for c in range(C):
        xt[c * NPAD:c * NPAD + NPC] = x[c * NPC:(c + 1) * NPC]
    safe2 = (t_pad + 1) // 2
    full = np.array([[128 * safe2, 128 * (t_pad - safe2)] * NTILE], np.int32)
    in_maps = []
    for c in range(C):
        nval_c = nval[c].reshape(1, -1) if trim else full
        m = {
            "xt_in": xt,
            "seg_in": np.ascontiguousarray(seg[c]),
            "gidx_in": gidx_w[c],
            "pool_in": np.ascontiguousarray(pool[c]),
            "id_in": ident,
            "ones_in": ones,
            "nval_in": nval_c,
        }
        for i in range(5):
            w = np.asarray(inputs[f"W{i + 1}"], dtype=np.float32)
            b = np.asarray(inputs[f"b{i + 1}"], dtype=np.float32)
            m[f"w{i}_in"] = w
            m[f"b{i}_in"] = np.ascontiguousarray(b[None, :])
        in_maps.append(m)
    return in_maps, t_pad, nchunk


def kernel(**inputs):
    in_maps, t_pad, nchunk = make_in_maps(inputs)
    nc = _get_program(t_pad, nchunk)
    res = bass_utils.run_bass_kernel_spmd(
        nc, in_maps, core_ids=list(range(C)))
    return res.results[0]["out"].astype(np.float32)
